# revision 25
# baseline (speedup 1.0000x reference)
"""Trainium2 Bass kernel for the 8x8-block rfft2 magnitude ("DCT") layer.

Computes, for input x [32,1,512,512] f32 and freq_weights [64] f32:
  per 8x8 spatial block: |rfft2(block, norm='ortho')| -> 40 freq bins,
  scaled by sigmoid(freq_weights)[:40], zero-padded to 64 channels.
Output: [32, 64, 64, 64] f32 (channels 40..63 are zero).

Strategy (pure data parallel, 4 images per core on 8 cores):
  The per-block 2D DFT is separable.  Per 128-row x 512-col slab:
    stage 1 (one f32r matmul per 128-col chunk): the data chunk is the
      *stationary* operand, a block-diagonal cos/sin matrix W1 streams:
      Z = A_chunk.T @ W1 -> vertical DFT of every row-block with the
      output transposed so j (intra-block col) is on partitions.
    stage 2 (two accumulating bf16 matmuls per chunk): Z re/im halves
      stationary (bf16 -> fast weight load), [C2|S2] / [-S2|C2] stream
      160 cols -> Fre|Fim of the 2D DFT in PSUM.
  Tail per slab: one ACT square PSUM->SBUF bf16, one re^2+im^2 add
  (GPSIMD by default), one ACT sqrt with the uniform sigmoid weight
  folded into the activation scale.  Output is stored in the
  *device-native* layout [img, slab, 128, 320] bf16 (640B+ contiguous
  runs per partition, at DMA line rate); the host permutes/casts to
  [B, 64, 64, 64] f32 and fills channels 40..63 with zeros.  This
  halves store traffic vs f32 NCHW and avoids its 256B-run RMW
  penalty.
  DMA schedule: ALL 16 per-slab (256KB) input loads are issued
  upfront on the SP HWDGE ring, so load issue never sits behind
  compute ops in an engine queue (the old kernel issued loads from the
  ACT queue, where they queued behind squares/sqrts); stores follow on
  the same ring, by which point no loads remain to head-of-line block.
  The ACT queue carries only compute.  Both x and out use a
  partition-major DRAM layout ([128, slab, w], host transposes) so
  every DMA is a contiguous >=512B-per-partition run -- no strided
  cross-row descriptors (those were both slow in the scheduler's cost
  model and, for the pair-store rearrange, produced corrupted data on
  hardware).  Per-DMA HWDGE descriptor-gen is ~0.6us serialized, so
  DMA count matters as much as bytes.
  PSUM: one [128,1024] stage-1 tile (2 banks) and one [128,1024]
  stage-2 tile per slab, double-buffered = all 8 banks; one big
  PSUM->SBUF DVE copy per slab (f32->bf16 cast) instead of two.
  Steady-state per-slab engine work (cost model): PE ~1.13us,
  DVE ~1.19us, ACT ~1.17us, Pool ~0.73us; slab pipeline is emitted in
  three staggered phases (stage-1 of slab k+1 ahead of stage-2 of
  slab k) with PSUM tiles explicitly rotated by slab parity.
  Measured (on-device repeat-loop slope, 8 cores concurrent):
  ~29.7us/invocation vs ~48.8us for the previous f32 kernel; engine
  busy (cost-model sim): ACT ~20us, DVE ~19us, PE ~18us, DMA ~16us.
  Session-to-session HW variance is ~+/-3us; config knobs in `cfg`
  were chosen by same-process A/B.
"""

import math
import numpy as np
from contextlib import ExitStack

import ml_dtypes
import concourse.bacc as bacc
import concourse.mybir as mybir
from concourse import tile
from concourse.bass_utils import run_bass_kernel_spmd

F32 = mybir.dt.float32
F32R = mybir.dt.float32r
BF16 = mybir.dt.bfloat16

N_CORES = 8
IMGS_PER_CORE = 4  # 32 / 8
SLABS_PER_IMG = 4  # 512 rows / 128


def _build_host_matrices(freq_weights: np.ndarray):
    """W1 f32 [128,256], CSB bf16 [128,320], Wtile bf16 [128,320]."""
    p = np.arange(128)
    # W1 [128, 256]: row p=(bi,i); col n=(reim, bi2, u). Vertical DFT, /8.
    bi_p, i_p = p // 8, p % 8
    n = np.arange(256)
    reim_n, r = n // 128, n % 128
    bi2_n, u_n = r // 8, r % 8
    ang1 = 2.0 * math.pi * np.outer(i_p, u_n) / 8.0
    W1 = np.where(reim_n[None, :] == 0, np.cos(ang1), np.sin(ang1)) / 8.0
    W1 *= (bi_p[:, None] == bi2_n[None, :])
    W1 = W1.astype(np.float32)

    # C2/S2 [128, 80]: row p=(bj,j); col m=(v, bj2). Horizontal DFT.
    bj_p, j_p = p // 8, p % 8
    m = np.arange(80)
    v_m, bj2_m = m // 16, m % 16
    ang2 = 2.0 * math.pi * np.outer(j_p, v_m) / 8.0
    blk = (bj_p[:, None] == bj2_m[None, :])
    C2 = (np.cos(ang2) * blk).astype(np.float32)
    S2 = (np.sin(ang2) * blk).astype(np.float32)
    # CSB [128, 320] bf16: [C2|S2] then [-S2|C2]
    CSB = np.concatenate(
        [C2, S2, -S2, C2], axis=1
    ).astype(ml_dtypes.bfloat16)

    # Wtile [128, 320]: p=(bi,u8), f=(c,v,q) -> sigmoid(freq_weights)[u*5+v]
    w = 1.0 / (1.0 + np.exp(-freq_weights.astype(np.float64)))
    u_idx = np.arange(128) % 8
    v_idx = (np.arange(320) // 16) % 5
    Wtile = w[u_idx[:, None] * 5 + v_idx[None, :]].astype(ml_dtypes.bfloat16)
    return W1, CSB, Wtile


_NC_CACHE = {}


def _build_bass(n_imgs: int = IMGS_PER_CORE, repeat: int = 1, cfg: dict = None):
    cfg = dict(cfg or {})
    add_eng = cfg.get("add", "pool")   # re^2+im^2 add: "pool" | "dve"
    wm_eng = cfg.get("wm", "pool")     # non-uniform weight mul engine
    zb = cfg.get("z", 12)
    sqb = cfg.get("sq", 8)
    magb = cfg.get("mag", 6)
    psz_b = cfg.get("psz", 2)
    pso_b = cfg.get("pso", 2)
    depth = cfg.get("depth", 0)
    uni_w = cfg.get("uniform_w")       # sigmoid value if weights uniform
    ew_f32 = cfg.get("ew_f32", 0)      # debug: f32 elementwise chain
    z_f32 = cfg.get("z_f32", 0)        # debug: f32r stage-2 (no explicit LDW)

    n_pairs = n_imgs * SLABS_PER_IMG // 2
    n_slabs_t = n_imgs * SLABS_PER_IMG
    nc = bacc.Bacc("TRN2", target_bir_lowering=False)
    # partition-major: x_pm[p, si, w] = image row 128*si+p, col w
    x = nc.dram_tensor(
        "x", [128, n_slabs_t, 512], F32R, kind="ExternalInput"
    )
    cstA = nc.dram_tensor("cstA", [128, 256], F32R, kind="ExternalInput")
    csb_cols = (512 if z_f32 else 320) if uni_w is not None else 640
    cstB = nc.dram_tensor(
        "cstB", [128, csb_cols], F32R if z_f32 else BF16, kind="ExternalInput"
    )
    out = nc.dram_tensor(
        "out", [128, n_slabs_t, 320], BF16, kind="ExternalOutput"
    )

    with tile.TileContext(nc) as tc, ExitStack() as ctx:
        consts = ctx.enter_context(tc.tile_pool(name="consts", bufs=1))
        a_pool = ctx.enter_context(tc.tile_pool(name="a", bufs=n_pairs))
        z_pool = ctx.enter_context(tc.tile_pool(name="z", bufs=zb))
        sq_pool = ctx.enter_context(tc.tile_pool(name="sq", bufs=sqb))
        mag_pool = ctx.enter_context(tc.tile_pool(name="mag", bufs=magb))
        psz_pool = ctx.enter_context(
            tc.tile_pool(name="psz", bufs=psz_b, space="PSUM")
        )
        pso_pool = ctx.enter_context(
            tc.tile_pool(name="pso", bufs=pso_b, space="PSUM")
        )

        w1_t = consts.tile([128, 256], F32R, tag="w1")
        csb_t = consts.tile([128, csb_cols], F32R if z_f32 else BF16, tag="csb")
        if z_f32:
            cs2_t = csb_t[:, 0:256]
            snc2_t = csb_t[:, 256:512]
        else:
            cs2_t = csb_t[:, 0:160]
            snc2_t = csb_t[:, 160:320]
        wt_t = csb_t[:, 320:640] if uni_w is None else None

        def emit_loads():
            """All input loads upfront on the SP ring, one DMA per slab
            pair (contiguous 4KB per partition in the partition-major x).
            Order: pair0, w1 (needed by the first matmul), csb, rest."""
            a_ts = [
                a_pool.tile([128, 1024], F32R, name="a_t")
                for p in range(n_pairs)
            ]

            ld_pair = cfg.get("ld_pair", 0)

            def load_pair(p):
                if ld_pair:
                    nc.sync.dma_start(
                        a_ts[p][:].rearrange("p (t w) -> p t w", t=2),
                        x[:, 2 * p : 2 * p + 2, :],
                    )
                else:
                    for h in range(2):
                        nc.sync.dma_start(
                            a_ts[p][:, 512 * h : 512 * (h + 1)],
                            x[:, 2 * p + h, :],
                        )

            # ramp: w1 first (tiny, needed by the very first matmul),
            # slab 0 split in two so stage-1 chunks 0-1 start ~1us
            # earlier, then everything else
            if cfg.get("ramp_split", 1):
                nc.sync.dma_start(w1_t[:], cstA[:])
                nc.sync.dma_start(
                    a_ts[0][:, 0:256], x[:, 0, 0:256]
                )
                nc.sync.dma_start(
                    a_ts[0][:, 256:512], x[:, 0, 256:512]
                )
                nc.sync.dma_start(a_ts[0][:, 512:1024], x[:, 1, :])
                nc.sync.dma_start(csb_t[:], cstB[:])
            else:
                load_pair(0)
                nc.sync.dma_start(w1_t[:], cstA[:])
                nc.sync.dma_start(csb_t[:], cstB[:])
            for p in range(1, n_pairs):
                load_pair(p)
            return a_ts

        a_ts = emit_loads()

        # warm the ACT function tables (Square, Sqrt) at t=0
        warm = consts.tile([128, 8], F32, tag="warm")
        nc.gpsimd.memset(warm[:], 0.0)
        nc.scalar.square(warm[:], warm[:])
        nc.scalar.sqrt(warm[:], warm[:])
        pe_warm = cfg.get("pe_warm", 0)

        # PSUM tiles allocated once and rotated by slab parity: reuse
        # distance is then exactly `bufs` slabs (the pool's stack
        # allocator would otherwise recycle the most-recent buffer and
        # serialize adjacent slabs).
        psz_t = [
            psz_pool.tile([128, 1024], F32, tag="psz", name=f"psz{i}")
            for i in range(psz_b)
        ]
        pso_t = [
            pso_pool.tile([128, 1024], F32, tag="o2", name=f"o2{i}")
            for i in range(pso_b)
        ]
        # spin the PE clock (HAM ramp) before real data arrives
        for _ in range(pe_warm):
            nc.tensor.matmul(
                psz_t[0][0:8, 0:8], warm[:], warm[:], start=True, stop=True
            )

        hp = cfg.get("hp", 0)

        sq_dt = F32 if ew_f32 else BF16
        z_dt = F32R if z_f32 else BF16
        s2_w = 256 if z_f32 else 160

        def emit_s1(a_half, psz):
            """Stage 1 (4 PE matmuls) + the PSUM->SBUF DVE evacuation."""
            import contextlib

            hpctx = (
                tc.high_priority(offset=hp if hp > 1 else None)
                if hp
                else contextlib.nullcontext()
            )
            with hpctx:
                for c in range(4):
                    nc.tensor.matmul(
                        psz[:, 256 * c : 256 * (c + 1)],
                        a_half[:, 128 * c : 128 * (c + 1)],
                        w1_t,
                        start=True,
                        stop=True,
                    )
                z_p = z_pool.tile([128, 1024], z_dt)
                nc.vector.tensor_copy(z_p[:], psz[:])
            return z_p

        def emit_s2(z_p, o2):
            """Stage 2 (8 PE matmuls) + one ACT square."""
            for c in range(4):
                zre = z_p[:, 256 * c : 256 * c + 128]
                zim = z_p[:, 256 * c + 128 : 256 * c + 256]
                dst = o2[:, 256 * c : 256 * c + s2_w]
                nc.tensor.matmul(dst, zre, cs2_t, start=True, stop=False)
                nc.tensor.matmul(dst, zim, snc2_t, start=False, stop=True)
            sq = sq_pool.tile([128, 640], sq_dt, tag="sq")
            # (square always reads the used 160-col slices)
            nc.scalar.square(
                sq[:].rearrange("p (c g) -> p c g", c=4, g=160),
                o2[:].rearrange("p (c w) -> p c w", c=4, w=256)[:, :, 0:160],
            )
            return sq

        def emit_add(sq, ssum_half):
            sqv = sq[:].rearrange("p (c r g) -> p c r g", c=4, r=2, g=80)
            add_fn = (
                nc.gpsimd.tensor_add
                if add_eng == "pool"
                else nc.vector.tensor_add
            )
            add_fn(
                ssum_half.rearrange("p (c g) -> p c g", c=4, g=80),
                sqv[:, :, 0],
                sqv[:, :, 1],
            )

        def emit_sqrt(ssum_ap, root_ap):
            if uni_w is not None:
                # uniform sigmoid weight w: w*sqrt(s) == sqrt(s*w^2)
                nc.scalar.activation(
                    root_ap,
                    ssum_ap,
                    mybir.ActivationFunctionType.Sqrt,
                    0.0,
                    float(uni_w) * float(uni_w),
                )
                return
            nc.scalar.sqrt(root_ap, ssum_ap)
            mul_fn = (
                nc.gpsimd.tensor_mul
                if wm_eng == "pool"
                else nc.vector.tensor_mul
            )
            mul_fn(root_ap, root_ap, wt_t)

        rep_ctx = tc.For_i(0, repeat, 1) if repeat > 1 else None
        if rep_ctx is not None:
            rep_ctx.__enter__()
        # 3-phase software pipeline, staggered in EMISSION order so the
        # strict per-engine FIFOs never head-of-line block: stage-1 of
        # slab k+lag2 is emitted before stage-2 of slab k (PE runs it
        # while DVE evacuates slab k), and tails lag stage-2 by `depth`
        # more slabs.  Every slab-pair shares one [128,640] root tile,
        # stored with a single DMA.
        lag2 = cfg.get("lag2", 1)
        n_slabs = 2 * n_pairs
        roots = {}
        zs, sqs = {}, {}

        sqrt_pair = cfg.get("sqrt_pair", 0)
        ssums = {}

        def do_tail(si):
            p, h = si // 2, si % 2
            if sqrt_pair:
                if h == 0:
                    ssums[p] = mag_pool.tile(
                        [128, 640], sq_dt, tag="ssum", name="ssum"
                    )
                emit_add(sqs.pop(si), ssums[p][:, 320 * h : 320 * (h + 1)])
                if h == 1:
                    root = mag_pool.tile(
                        [128, 640], BF16, tag="root", name="root"
                    )
                    emit_sqrt(ssums[p][:], root[:])
                    nc.sync.dma_start(
                        out[:, si - 1 : si + 1, :],
                        root[:].rearrange("p (t w) -> p t w", t=2),
                    )
            else:
                ssum = mag_pool.tile(
                    [128, 320], sq_dt, tag="ssum", name="ssum"
                )
                emit_add(sqs.pop(si), ssum[:])
                root = mag_pool.tile(
                    [128, 320], BF16, tag="root", name="root"
                )
                emit_sqrt(ssum[:], root[:])
                nc.sync.dma_start(out[:, si, :], root[:])

        for si in range(n_slabs + lag2 + depth):
            if si < n_slabs:
                p, h = si // 2, si % 2
                zs[si] = emit_s1(
                    a_ts[p][:, 512 * h : 512 * (h + 1)],
                    psz_t[si % psz_b],
                )
            if lag2 <= si < n_slabs + lag2:
                k = si - lag2
                sqs[k] = emit_s2(zs.pop(k), pso_t[k % pso_b])
            if si >= lag2 + depth:
                do_tail(si - lag2 - depth)
        if rep_ctx is not None:
            rep_ctx.__exit__(None, None, None)
    nc.finalize()
    return nc


def _make_inputs(x: np.ndarray, freq_weights: np.ndarray, z_f32: bool = False):
    """Full inputs -> (uniform_w, per-core in_maps)."""
    W1, CSB, Wtile = _build_host_matrices(freq_weights)
    if z_f32:
        c = CSB.astype(np.float32)
        z96 = np.zeros((128, 96), np.float32)
        CSB = np.concatenate([c[:, 0:160], z96, c[:, 160:320], z96], 1)
    uni = None
    if np.all(freq_weights == freq_weights[0]):
        uni = float(1.0 / (1.0 + np.exp(-float(freq_weights[0]))))
        cstB = CSB
    else:
        cstB = np.concatenate([CSB, Wtile], axis=1)
    per = x.shape[0] // N_CORES
    # partition-major: x_pm[core][p, si, w] = x[core*per + si//4, 0,
    #   128*(si%4) + p, w]
    xr = np.ascontiguousarray(
        x.reshape(N_CORES, per * 4, 128, 512).transpose(0, 2, 1, 3)
    )
    in_maps = [
        {"x": xr[k], "cstA": W1, "cstB": cstB} for k in range(N_CORES)
    ]
    return uni, in_maps


def _postprocess(core_outs) -> np.ndarray:
    """[n_cores] of [128, 16, 320] bf16 -> [32, 64, 64, 64] f32."""
    raw = np.stack([np.asarray(o) for o in core_outs], axis=0)
    B = raw.shape[0] * raw.shape[2] // SLABS_PER_IMG
    # [core, (bi,u), img*s, (c,v,q)] -> [b, u, v, s, bi, c, q]
    a = raw.reshape(8, 16, 8, 4, 4, 4, 5, 16).astype(np.float32)
    a = a.transpose(0, 3, 2, 6, 4, 1, 5, 7).reshape(B, 40, 64, 64)
    out = np.zeros((B, 64, 64, 64), dtype=np.float32)
    out[:, :40] = a
    return out


def kernel(x: np.ndarray, freq_weights: np.ndarray) -> np.ndarray:
    x = np.ascontiguousarray(np.asarray(x, dtype=np.float32))
    freq_weights = np.asarray(freq_weights, dtype=np.float32)
    assert x.shape == (32, 1, 512, 512) and freq_weights.shape == (64,)

    uni, in_maps = _make_inputs(x, freq_weights)
    if uni not in _NC_CACHE:
        _NC_CACHE[uni] = _build_bass(cfg={"uniform_w": uni})
    nc = _NC_CACHE[uni]
    res = run_bass_kernel_spmd(nc, in_maps, list(range(N_CORES))).results
    return _postprocess([res[k]["out"] for k in range(N_CORES)])


# revision 27
# speedup vs baseline: 1.1954x; 1.1954x over previous
"""Trainium2 Bass kernel for the 8x8-block rfft2 magnitude ("DCT") layer.

Computes, for input x [32,1,512,512] f32 and freq_weights [64] f32:
  per 8x8 spatial block: |rfft2(block, norm='ortho')| -> 40 freq bins,
  scaled by sigmoid(freq_weights)[:40], zero-padded to 64 channels.
Output: [32, 64, 64, 64] f32 (channels 40..63 are zero).

Strategy (pure data parallel, 4 images per core on 8 cores):
  The per-block 2D DFT is separable.  Per 128-row x 512-col slab:
    stage 1 (one f32r matmul per 128-col chunk): the data chunk is the
      *stationary* operand, a block-diagonal cos/sin matrix W1 streams:
      Z = A_chunk.T @ W1 -> vertical DFT of every row-block with the
      output transposed so j (intra-block col) is on partitions.
    stage 2 (two accumulating bf16 matmuls per chunk): Z re/im halves
      stationary (bf16 -> fast weight load), [C2|S2] / [-S2|C2] stream
      160 cols -> Fre|Fim of the 2D DFT in PSUM.
  Tail per slab: one ACT square PSUM->SBUF bf16, one re^2+im^2 add
  (GPSIMD by default), one ACT sqrt with the uniform sigmoid weight
  folded into the activation scale.  Output is stored in the
  *device-native* layout [img, slab, 128, 320] bf16 (640B+ contiguous
  runs per partition, at DMA line rate); the host permutes/casts to
  [B, 64, 64, 64] f32 and fills channels 40..63 with zeros.  This
  halves store traffic vs f32 NCHW and avoids its 256B-run RMW
  penalty.
  DMA schedule: ALL 16 per-slab (256KB) input loads are issued
  upfront on the SP HWDGE ring, so load issue never sits behind
  compute ops in an engine queue (the old kernel issued loads from the
  ACT queue, where they queued behind squares/sqrts); stores follow on
  the same ring, by which point no loads remain to head-of-line block.
  The ACT queue carries only compute.  Both x and out use a
  partition-major DRAM layout ([128, slab, w], host transposes) so
  every DMA is a contiguous >=512B-per-partition run -- no strided
  cross-row descriptors (those were both slow in the scheduler's cost
  model and, for the pair-store rearrange, produced corrupted data on
  hardware).  Per-DMA HWDGE descriptor-gen is ~0.6us serialized, so
  DMA count matters as much as bytes.
  PSUM: one [128,1024] stage-1 tile (2 banks) and one [128,1024]
  stage-2 tile per slab, double-buffered = all 8 banks; one big
  PSUM->SBUF DVE copy per slab (f32->bf16 cast) instead of two.
  Steady-state per-slab engine work (cost model): PE ~1.13us,
  DVE ~1.19us, ACT ~1.17us, Pool ~0.73us; slab pipeline is emitted in
  three staggered phases (stage-1 of slab k+1 ahead of stage-2 of
  slab k) with PSUM tiles explicitly rotated by slab parity.
  Measured (on-device repeat-loop slope, 8 cores concurrent):
  ~29.7us/invocation vs ~48.8us for the previous f32 kernel; engine
  busy (cost-model sim): ACT ~20us, DVE ~19us, PE ~18us, DMA ~16us.
  Session-to-session HW variance is ~+/-3us; config knobs in `cfg`
  were chosen by same-process A/B.
"""

import math
import numpy as np
from contextlib import ExitStack

import ml_dtypes
import concourse.bacc as bacc
import concourse.mybir as mybir
from concourse import tile
from concourse.bass_utils import run_bass_kernel_spmd

F32 = mybir.dt.float32
F32R = mybir.dt.float32r
BF16 = mybir.dt.bfloat16

N_CORES = 8
IMGS_PER_CORE = 4  # 32 / 8
SLABS_PER_IMG = 4  # 512 rows / 128


def _build_host_matrices(freq_weights: np.ndarray):
    """W1 f32 [128,256], CSB bf16 [128,320], Wtile bf16 [128,320]."""
    p = np.arange(128)
    # W1 [128, 256]: row p=(bi,i); col n=(reim, bi2, u). Vertical DFT, /8.
    bi_p, i_p = p // 8, p % 8
    n = np.arange(256)
    reim_n, r = n // 128, n % 128
    bi2_n, u_n = r // 8, r % 8
    ang1 = 2.0 * math.pi * np.outer(i_p, u_n) / 8.0
    W1 = np.where(reim_n[None, :] == 0, np.cos(ang1), np.sin(ang1)) / 8.0
    W1 *= (bi_p[:, None] == bi2_n[None, :])
    W1 = W1.astype(np.float32)

    # C2/S2 [128, 80]: row p=(bj,j); col m=(v, bj2). Horizontal DFT.
    bj_p, j_p = p // 8, p % 8
    m = np.arange(80)
    v_m, bj2_m = m // 16, m % 16
    ang2 = 2.0 * math.pi * np.outer(j_p, v_m) / 8.0
    blk = (bj_p[:, None] == bj2_m[None, :])
    C2 = (np.cos(ang2) * blk).astype(np.float32)
    S2 = (np.sin(ang2) * blk).astype(np.float32)
    # CSB [128, 320] bf16: [C2|S2] then [-S2|C2]
    CSB = np.concatenate(
        [C2, S2, -S2, C2], axis=1
    ).astype(ml_dtypes.bfloat16)

    # Wtile [128, 320]: p=(bi,u8), f=(c,v,q) -> sigmoid(freq_weights)[u*5+v]
    w = 1.0 / (1.0 + np.exp(-freq_weights.astype(np.float64)))
    u_idx = np.arange(128) % 8
    v_idx = (np.arange(320) // 16) % 5
    Wtile = w[u_idx[:, None] * 5 + v_idx[None, :]].astype(ml_dtypes.bfloat16)
    return W1, CSB, Wtile


_NC_CACHE = {}


def _build_bass(n_imgs: int = IMGS_PER_CORE, repeat: int = 1, cfg: dict = None):
    cfg = dict(cfg or {})
    add_eng = cfg.get("add", "pool")   # re^2+im^2 add: "pool" | "dve"
    wm_eng = cfg.get("wm", "pool")     # non-uniform weight mul engine
    zb = cfg.get("z", 12)
    sqb = cfg.get("sq", 8)
    magb = cfg.get("mag", 6)
    psz_b = cfg.get("psz", 2)
    pso_b = cfg.get("pso", 2)
    depth = cfg.get("depth", 0)
    uni_w = cfg.get("uniform_w")       # sigmoid value if weights uniform
    ew_f32 = cfg.get("ew_f32", 0)      # debug: f32 elementwise chain
    z_f32 = cfg.get("z_f32", 0)        # debug: f32r stage-2 (no explicit LDW)

    n_pairs = n_imgs * SLABS_PER_IMG // 2
    n_slabs_t = n_imgs * SLABS_PER_IMG
    nc = bacc.Bacc("TRN2", target_bir_lowering=False)
    # partition-major: x_pm[p, si, w] = image row 128*si+p, col w
    x = nc.dram_tensor(
        "x", [128, n_slabs_t, 512], F32R, kind="ExternalInput"
    )
    cstA = nc.dram_tensor("cstA", [128, 256], F32R, kind="ExternalInput")
    csb_cols = (512 if z_f32 else 320) if uni_w is not None else 640
    cstB = nc.dram_tensor(
        "cstB", [128, csb_cols], F32R if z_f32 else BF16, kind="ExternalInput"
    )
    out = nc.dram_tensor(
        "out", [128, n_slabs_t, 320], BF16, kind="ExternalOutput"
    )

    with tile.TileContext(nc) as tc, ExitStack() as ctx:
        consts = ctx.enter_context(tc.tile_pool(name="consts", bufs=1))
        a_pool = ctx.enter_context(tc.tile_pool(name="a", bufs=n_pairs))
        z_pool = ctx.enter_context(tc.tile_pool(name="z", bufs=zb))
        sq_pool = ctx.enter_context(tc.tile_pool(name="sq", bufs=sqb))
        mag_pool = ctx.enter_context(tc.tile_pool(name="mag", bufs=magb))
        psz_pool = ctx.enter_context(
            tc.tile_pool(name="psz", bufs=psz_b, space="PSUM")
        )
        pso_pool = ctx.enter_context(
            tc.tile_pool(name="pso", bufs=pso_b, space="PSUM")
        )

        w1_t = consts.tile([128, 256], F32R, tag="w1")
        csb_t = consts.tile([128, csb_cols], F32R if z_f32 else BF16, tag="csb")
        if z_f32:
            cs2_t = csb_t[:, 0:256]
            snc2_t = csb_t[:, 256:512]
        else:
            cs2_t = csb_t[:, 0:160]
            snc2_t = csb_t[:, 160:320]
        wt_t = csb_t[:, 320:640] if uni_w is None else None

        def emit_loads():
            """All input loads upfront on the SP ring, one DMA per slab
            pair (contiguous 4KB per partition in the partition-major x).
            Order: pair0, w1 (needed by the first matmul), csb, rest."""
            a_ts = [
                a_pool.tile([128, 1024], F32R, name="a_t")
                for p in range(n_pairs)
            ]

            ld_pair = cfg.get("ld_pair", 0)

            def load_pair(p):
                if ld_pair:
                    nc.sync.dma_start(
                        a_ts[p][:].rearrange("p (t w) -> p t w", t=2),
                        x[:, 2 * p : 2 * p + 2, :],
                    )
                else:
                    for h in range(2):
                        nc.sync.dma_start(
                            a_ts[p][:, 512 * h : 512 * (h + 1)],
                            x[:, 2 * p + h, :],
                        )

            # ramp: w1 first (tiny, needed by the very first matmul),
            # slab 0 split in two so stage-1 chunks 0-1 start ~1us
            # earlier, then everything else
            if cfg.get("ramp_split", 1 if repeat == 1 else 0):
                nc.sync.dma_start(w1_t[:], cstA[:])
                nc.sync.dma_start(
                    a_ts[0][:, 0:256], x[:, 0, 0:256]
                )
                nc.sync.dma_start(
                    a_ts[0][:, 256:512], x[:, 0, 256:512]
                )
                nc.sync.dma_start(a_ts[0][:, 512:1024], x[:, 1, :])
                nc.sync.dma_start(csb_t[:], cstB[:])
            else:
                load_pair(0)
                nc.sync.dma_start(w1_t[:], cstA[:])
                nc.sync.dma_start(csb_t[:], cstB[:])
            for p in range(1, n_pairs):
                load_pair(p)
            return a_ts

        a_ts = emit_loads()

        # warm the ACT function tables (Square, Sqrt) at t=0
        warm = consts.tile([128, 8], F32, tag="warm")
        nc.gpsimd.memset(warm[:], 0.0)
        nc.scalar.square(warm[:], warm[:])
        nc.scalar.sqrt(warm[:], warm[:])
        pe_warm = cfg.get("pe_warm", 0)

        # PSUM tiles allocated once and rotated by slab parity: reuse
        # distance is then exactly `bufs` slabs (the pool's stack
        # allocator would otherwise recycle the most-recent buffer and
        # serialize adjacent slabs).
        psz_t = [
            psz_pool.tile([128, 1024], F32, tag="psz", name=f"psz{i}")
            for i in range(psz_b)
        ]
        pso_t = [
            pso_pool.tile([128, 1024], F32, tag="o2", name=f"o2{i}")
            for i in range(pso_b)
        ]
        # spin the PE clock (HAM ramp) before real data arrives
        for _ in range(pe_warm):
            nc.tensor.matmul(
                psz_t[0][0:8, 0:8], warm[:], warm[:], start=True, stop=True
            )

        hp = cfg.get("hp", 0)

        sq_dt = F32 if ew_f32 else BF16
        z_dt = F32R if z_f32 else BF16
        s2_w = 256 if z_f32 else 160

        def emit_s1(a_half, psz):
            """Stage 1 (4 PE matmuls) + the PSUM->SBUF DVE evacuation."""
            import contextlib

            hpctx = (
                tc.high_priority(offset=hp if hp > 1 else None)
                if hp
                else contextlib.nullcontext()
            )
            with hpctx:
                for c in range(4):
                    nc.tensor.matmul(
                        psz[:, 256 * c : 256 * (c + 1)],
                        a_half[:, 128 * c : 128 * (c + 1)],
                        w1_t,
                        start=True,
                        stop=True,
                    )
                z_p = z_pool.tile([128, 1024], z_dt)
                if cfg.get("cop_split", 0):
                    nc.vector.tensor_copy(z_p[:, 0:512], psz[:, 0:512])
                    nc.vector.tensor_copy(z_p[:, 512:1024], psz[:, 512:1024])
                else:
                    nc.vector.tensor_copy(z_p[:], psz[:])
            return z_p

        def emit_s2(z_p, o2):
            """Stage 2 (8 PE matmuls) + one ACT square."""
            for c in range(4):
                zre = z_p[:, 256 * c : 256 * c + 128]
                zim = z_p[:, 256 * c + 128 : 256 * c + 256]
                dst = o2[:, 256 * c : 256 * c + s2_w]
                nc.tensor.matmul(dst, zre, cs2_t, start=True, stop=False)
                nc.tensor.matmul(dst, zim, snc2_t, start=False, stop=True)
            sq = sq_pool.tile([128, 640], sq_dt, tag="sq")
            # (square always reads the used 160-col slices)
            nc.scalar.square(
                sq[:].rearrange("p (c g) -> p c g", c=4, g=160),
                o2[:].rearrange("p (c w) -> p c w", c=4, w=256)[:, :, 0:160],
            )
            return sq

        def emit_add(sq, ssum_half):
            sqv = sq[:].rearrange("p (c r g) -> p c r g", c=4, r=2, g=80)
            add_fn = (
                nc.gpsimd.tensor_add
                if add_eng == "pool"
                else nc.vector.tensor_add
            )
            add_fn(
                ssum_half.rearrange("p (c g) -> p c g", c=4, g=80),
                sqv[:, :, 0],
                sqv[:, :, 1],
            )

        def emit_sqrt(ssum_ap, root_ap):
            if uni_w is not None:
                # uniform sigmoid weight w: w*sqrt(s) == sqrt(s*w^2)
                nc.scalar.activation(
                    root_ap,
                    ssum_ap,
                    mybir.ActivationFunctionType.Sqrt,
                    0.0,
                    float(uni_w) * float(uni_w),
                )
                return
            nc.scalar.sqrt(root_ap, ssum_ap)
            mul_fn = (
                nc.gpsimd.tensor_mul
                if wm_eng == "pool"
                else nc.vector.tensor_mul
            )
            mul_fn(root_ap, root_ap, wt_t)

        rep_ctx = tc.For_i(0, repeat, 1) if repeat > 1 else None
        if rep_ctx is not None:
            rep_ctx.__enter__()
        # 3-phase software pipeline, staggered in EMISSION order so the
        # strict per-engine FIFOs never head-of-line block: stage-1 of
        # slab k+lag2 is emitted before stage-2 of slab k (PE runs it
        # while DVE evacuates slab k), and tails lag stage-2 by `depth`
        # more slabs.  Every slab-pair shares one [128,640] root tile,
        # stored with a single DMA.
        lag2 = cfg.get("lag2", 1)
        n_slabs = 2 * n_pairs
        roots = {}
        zs, sqs = {}, {}

        sqrt_pair = cfg.get("sqrt_pair", 0)
        ssums = {}

        def do_tail(si):
            p, h = si // 2, si % 2
            if sqrt_pair:
                if h == 0:
                    ssums[p] = mag_pool.tile(
                        [128, 640], sq_dt, tag="ssum", name="ssum"
                    )
                emit_add(sqs.pop(si), ssums[p][:, 320 * h : 320 * (h + 1)])
                if h == 1:
                    root = mag_pool.tile(
                        [128, 640], BF16, tag="root", name="root"
                    )
                    emit_sqrt(ssums[p][:], root[:])
                    nc.sync.dma_start(
                        out[:, si - 1 : si + 1, :],
                        root[:].rearrange("p (t w) -> p t w", t=2),
                    )
            else:
                ssum = mag_pool.tile(
                    [128, 320], sq_dt, tag="ssum", name="ssum"
                )
                emit_add(sqs.pop(si), ssum[:])
                root = mag_pool.tile(
                    [128, 320], BF16, tag="root", name="root"
                )
                emit_sqrt(ssum[:], root[:])
                nc.sync.dma_start(out[:, si, :], root[:])

        for si in range(n_slabs + lag2 + depth):
            if si < n_slabs:
                p, h = si // 2, si % 2
                zs[si] = emit_s1(
                    a_ts[p][:, 512 * h : 512 * (h + 1)],
                    psz_t[si % psz_b],
                )
            if lag2 <= si < n_slabs + lag2:
                k = si - lag2
                sqs[k] = emit_s2(zs.pop(k), pso_t[k % pso_b])
            if si >= lag2 + depth:
                do_tail(si - lag2 - depth)
        if rep_ctx is not None:
            rep_ctx.__exit__(None, None, None)
    nc.finalize()
    return nc


def _make_inputs(x: np.ndarray, freq_weights: np.ndarray, z_f32: bool = False):
    """Full inputs -> (uniform_w, per-core in_maps)."""
    W1, CSB, Wtile = _build_host_matrices(freq_weights)
    if z_f32:
        c = CSB.astype(np.float32)
        z96 = np.zeros((128, 96), np.float32)
        CSB = np.concatenate([c[:, 0:160], z96, c[:, 160:320], z96], 1)
    uni = None
    if np.all(freq_weights == freq_weights[0]):
        uni = float(1.0 / (1.0 + np.exp(-float(freq_weights[0]))))
        cstB = CSB
    else:
        cstB = np.concatenate([CSB, Wtile], axis=1)
    per = x.shape[0] // N_CORES
    # partition-major: x_pm[core][p, si, w] = x[core*per + si//4, 0,
    #   128*(si%4) + p, w]
    xr = np.ascontiguousarray(
        x.reshape(N_CORES, per * 4, 128, 512).transpose(0, 2, 1, 3)
    )
    in_maps = [
        {"x": xr[k], "cstA": W1, "cstB": cstB} for k in range(N_CORES)
    ]
    return uni, in_maps


def _postprocess(core_outs) -> np.ndarray:
    """[n_cores] of [128, 16, 320] bf16 -> [32, 64, 64, 64] f32."""
    raw = np.stack([np.asarray(o) for o in core_outs], axis=0)
    B = raw.shape[0] * raw.shape[2] // SLABS_PER_IMG
    # [core, (bi,u), img*s, (c,v,q)] -> [b, u, v, s, bi, c, q]
    a = raw.reshape(8, 16, 8, 4, 4, 4, 5, 16).astype(np.float32)
    a = a.transpose(0, 3, 2, 6, 4, 1, 5, 7).reshape(B, 40, 64, 64)
    out = np.zeros((B, 64, 64, 64), dtype=np.float32)
    out[:, :40] = a
    return out


def kernel(x: np.ndarray, freq_weights: np.ndarray) -> np.ndarray:
    x = np.ascontiguousarray(np.asarray(x, dtype=np.float32))
    freq_weights = np.asarray(freq_weights, dtype=np.float32)
    assert x.shape == (32, 1, 512, 512) and freq_weights.shape == (64,)

    uni, in_maps = _make_inputs(x, freq_weights)
    if uni not in _NC_CACHE:
        _NC_CACHE[uni] = _build_bass(cfg={"uniform_w": uni})
    nc = _NC_CACHE[uni]
    res = run_bass_kernel_spmd(nc, in_maps, list(range(N_CORES))).results
    return _postprocess([res[k]["out"] for k in range(N_CORES)])
